# revision 52
# baseline (speedup 1.0000x reference)
"""MLA (multi-head latent attention) Trainium2 kernel, 8-core SPMD.

Sharding: tensor-parallel over heads (16 heads / 8 cores = 2 heads per core).
The low-rank down-projections (c_q, c_kv, rotated k-rope) are computed
data-parallel over sequence positions (each core does its S/8 slice), then
all-gathered on-chip (two AllGather collectives so the kv-dependent
up-projections overlap the q-latent gather).  Up-projections, attention, and
the out-projection are head-sharded; each core produces a partial (S, E)
output (its 2 heads' contribution through wo) and the host sums the 8
partials.

All device matmuls run in bf16 with f32 PSUM accumulation.  Attention is
computed in transposed layout (logits[t, s], t on partitions) so that:
  - Q/K/V all come out of the projection matmuls in the right layout with no
    transposes anywhere,
  - softmax denominator Z[s] = sum_t exp(logits[t,s]) comes from two 2:1 DVE
    pre-reductions of the exp'd tiles followed by a short ones-vector matmul;
    1/Z is broadcast over partitions with a K=1 bf16 matmul,
  - exp is applied without max-subtraction (logits are O(6) here; reference
    softmax is mathematically identical).
RoPE's rotate-half permutation is folded into host-permuted copies of the
rope projection weights (an extra accumulation pass on the PE), and the
shared k-rope is duplicated across both head slots via host-duplicated
weights so per-head attention APs stay base-partition aligned.
"""

import sys
from contextlib import ExitStack

import numpy as np

sys.path.insert(0, "/opt/trn_rl_repo")

import ml_dtypes  # noqa: E402

import concourse.bass as bass  # noqa: E402
import concourse.tile as tile  # noqa: E402
from concourse import bacc  # noqa: E402
from concourse import mybir  # noqa: E402
from concourse.bass_utils import run_bass_kernel_spmd  # noqa: E402

BF16 = ml_dtypes.bfloat16
F32 = np.float32

# Model dims (hardcoded per problem spec)
E = 2048
H = 16
QR = 512
KVR = 512
DR = 64
DV = 128
S = 2048
NCORES = 8
HC = H // NCORES          # heads per core = 2
DQK = DV + DR             # 192
SCALE = 1.0 / float(np.sqrt(DQK))

P = 128                   # SBUF partitions
NCH = 4                   # s-chunks of 512
CH = S // NCH             # 512
KT = E // P               # 16 k-tiles over E
RT = QR // P              # 4 k-tiles over QR/KVR
TT = S // P               # 16 t-tiles over S
XS = S // NCORES          # per-core s-slice for the latent stage = 256
BPC = CH // XS            # gather blocks per s-chunk = 2

bf16 = mybir.dt.bfloat16
f32 = mybir.dt.float32


def _build_nc(repeat=1):
    nc = bacc.Bacc(None, target_bir_lowering=False)

    xTs = nc.declare_dram_parameter("xTs", [E, XS], bf16, isOutput=False)
    wqdT = nc.declare_dram_parameter("wqdT", [E, QR], bf16, isOutput=False)
    wkvdT = nc.declare_dram_parameter("wkvdT", [E, KVR], bf16, isOutput=False)
    # k-rope weights duplicated across both head slots (so per-head attention
    # APs stay base-partition aligned) + column-permuted copies (rotate-half)
    wkrT2 = nc.declare_dram_parameter("wkrT2", [E, HC * DR], bf16, isOutput=False)
    wkrT2p = nc.declare_dram_parameter("wkrT2p", [E, HC * DR], bf16, isOutput=False)
    wquT = nc.declare_dram_parameter("wquT", [QR, HC * DV], bf16, isOutput=False)
    wqrT = nc.declare_dram_parameter("wqrT", [QR, HC * DR], bf16, isOutput=False)
    wqrTp = nc.declare_dram_parameter("wqrTp", [QR, HC * DR], bf16, isOutput=False)
    wkuT = nc.declare_dram_parameter("wkuT", [KVR, HC * DV], bf16, isOutput=False)
    wvuT = nc.declare_dram_parameter("wvuT", [KVR, HC * DV], bf16, isOutput=False)
    woT = nc.declare_dram_parameter("woT", [HC * DV, E], bf16, isOutput=False)
    cos2 = nc.declare_dram_parameter("cos2", [HC * DR, S], bf16, isOutput=False)
    sin2 = nc.declare_dram_parameter("sin2", [HC * DR, S], bf16, isOutput=False)
    cosk = nc.declare_dram_parameter("cosk", [HC * DR, XS], bf16, isOutput=False)
    sink = nc.declare_dram_parameter("sink", [HC * DR, XS], bf16, isOutput=False)
    out = nc.declare_dram_parameter("out", [S, E], f32, isOutput=True)

    with tile.TileContext(nc) as tc, ExitStack() as ctx:
        const = ctx.enter_context(tc.tile_pool(name="const", bufs=1))
        wdp = ctx.enter_context(tc.tile_pool(name="wdown", bufs=1))
        wup = ctx.enter_context(tc.tile_pool(name="wup", bufs=1))
        xtp = ctx.enter_context(tc.tile_pool(name="xt", bufs=1))
        s1p = ctx.enter_context(tc.tile_pool(name="s1", bufs=2))
        latp = ctx.enter_context(tc.tile_pool(name="lat", bufs=2))
        pers = ctx.enter_context(tc.tile_pool(name="pers", bufs=1))
        ropep = ctx.enter_context(tc.tile_pool(name="rope", bufs=1))
        ppp = ctx.enter_context(tc.tile_pool(name="pp", bufs=2))
        zlp = ctx.enter_context(tc.tile_pool(name="zl", bufs=1))
        outp = ctx.enter_context(tc.tile_pool(name="outsb", bufs=1))
        recp = ctx.enter_context(tc.tile_pool(name="rec", bufs=2))
        dramp = ctx.enter_context(tc.tile_pool(name="dram", bufs=1, space="DRAM"))
        psA = ctx.enter_context(tc.tile_pool(name="psA", bufs=4, space="PSUM"))
        psB = ctx.enter_context(tc.tile_pool(name="psB", bufs=2, space="PSUM"))
        psZ = ctx.enter_context(tc.tile_pool(name="psZ", bufs=1, space="PSUM"))
        psC = ctx.enter_context(tc.tile_pool(name="psC", bufs=1, space="PSUM"))

        # --- constants ---
        ones = const.tile([P, 1], bf16)
        nc.any.memset(ones[:], 1.0)
        ones1f = const.tile([1, P], bf16)
        nc.any.memset(ones1f[:], 1.0)
        cos_sb = const.tile([P, S], bf16)
        nc.sync.dma_start(out=cos_sb[:], in_=cos2[:])
        sin_sb = const.tile([P, S], bf16)
        nc.sync.dma_start(out=sin_sb[:], in_=sin2[:])
        cosk_sb = const.tile([P, XS], bf16)
        nc.sync.dma_start(out=cosk_sb[:], in_=cosk[:])
        sink_sb = const.tile([P, XS], bf16)
        nc.sync.dma_start(out=sink_sb[:], in_=sink[:])

        # --- weights resident in SBUF ---
        wqd_sb = wdp.tile([P, KT, QR], bf16)
        nc.sync.dma_start(out=wqd_sb[:], in_=wqdT[:].rearrange("(k p) m -> p k m", p=P))
        wkvd_sb = wdp.tile([P, KT, KVR], bf16)
        nc.sync.dma_start(out=wkvd_sb[:], in_=wkvdT[:].rearrange("(k p) m -> p k m", p=P))
        wkr2_sb = wdp.tile([P, KT, HC * DR], bf16)
        nc.sync.dma_start(out=wkr2_sb[:], in_=wkrT2[:].rearrange("(k p) m -> p k m", p=P))
        wkr2p_sb = wdp.tile([P, KT, HC * DR], bf16)
        nc.sync.dma_start(out=wkr2p_sb[:], in_=wkrT2p[:].rearrange("(k p) m -> p k m", p=P))

        wqu_sb = wup.tile([P, RT, HC * DV], bf16)
        nc.sync.dma_start(out=wqu_sb[:], in_=wquT[:].rearrange("(k p) m -> p k m", p=P))
        wqr_sb = wup.tile([P, RT, HC * DR], bf16)
        nc.sync.dma_start(out=wqr_sb[:], in_=wqrT[:].rearrange("(k p) m -> p k m", p=P))
        wqrp_sb = wup.tile([P, RT, HC * DR], bf16)
        nc.sync.dma_start(out=wqrp_sb[:], in_=wqrTp[:].rearrange("(k p) m -> p k m", p=P))
        wku_sb = wup.tile([P, RT, HC * DV], bf16)
        nc.sync.dma_start(out=wku_sb[:], in_=wkuT[:].rearrange("(k p) m -> p k m", p=P))
        wvu_sb = wup.tile([P, RT, HC * DV], bf16)
        nc.sync.dma_start(out=wvu_sb[:], in_=wvuT[:].rearrange("(k p) m -> p k m", p=P))
        wo_sb = wup.tile([P, HC, E], bf16)
        nc.sync.dma_start(out=wo_sb[:], in_=woT[:].rearrange("(h p) e -> p h e", p=P))

        for _rep in range(repeat):
            _emit_iteration(
                nc, tc, ctx,
                xTs, out,
                ones, ones1f, cos_sb, sin_sb, cosk_sb, sink_sb,
                wqd_sb, wkvd_sb, wkr2_sb, wkr2p_sb,
                wqu_sb, wqr_sb, wqrp_sb, wku_sb, wvu_sb, wo_sb,
                xtp, s1p, latp, pers, ropep, ppp, zlp, outp, recp, dramp,
                psA, psB, psZ, psC,
            )

    nc.finalize()
    return nc


def _emit_iteration(nc, tc, ctx, xTs, out,
                    ones, ones1f, cos_sb, sin_sb, cosk_sb, sink_sb,
                    wqd_sb, wkvd_sb, wkr2_sb, wkr2p_sb,
                    wqu_sb, wqr_sb, wqrp_sb, wku_sb, wvu_sb, wo_sb,
                    xtp, s1p, latp, pers, ropep, ppp, zlp, outp, recp, dramp,
                    psA, psB, psZ, psC):
    # --- persistent activations (per iteration) ---
    qTc = pers.tile([P, HC, S], bf16, tag="qTc")    # q content, (d, h, s)
    qTr = pers.tile([P, S], bf16, tag="qTr")        # q rope rotated, (h*64+d, s)
    kTc = pers.tile([P, HC, S], bf16, tag="kTc")    # k content, (d, h, t)
    kTr2 = pers.tile([P, S], bf16, tag="kTr2")      # k rope rotated, dup per head
    vv = pers.tile([P, TT, HC * DV], bf16, tag="vv")  # v, (t%128, t//128, h*128+d)
    attnT = pers.tile([P, HC, S], bf16, tag="attnT")  # attention out, (d, h, s)

    lat_kv_in = dramp.tile([KVR + P, XS], bf16, tag="lkvi")
    lat_kv_out = dramp.tile([NCORES, KVR + P, XS], bf16, tag="lkvo",
                            addr_space="Shared")
    lat_q_in = dramp.tile([QR, XS], bf16, tag="lqi")
    lat_q_out = dramp.tile([NCORES, QR, XS], bf16, tag="lqo", addr_space="Shared")

    def rope_mix(dst, raw, rawp, csb, ssb, sl):
        """dst = raw * cos + perm(raw) * sin_signed (perm via rawp)."""
        n = sl.stop - sl.start
        t1 = ropep.tile([P, n], bf16, tag=f"ropet1_{n}")
        nc.vector.tensor_mul(t1[:], rawp[:], ssb[:, sl])
        t2 = ropep.tile([P, n], bf16, tag=f"ropet2_{n}")
        nc.vector.tensor_mul(t2[:], raw[:], csb[:, sl])
        nc.vector.tensor_add(dst, t1[:], t2[:])

    # ---- stage 1: latents for OWN s-slice, then all-gather ----
    xs = xtp.tile([P, KT, XS], bf16, tag="xs")
    nc.sync.dma_start(out=xs[:], in_=xTs[:].rearrange("(k p) n -> p k n", p=P))

    # c_kv slice
    for mt in range(RT):
        ps = psA.tile([P, CH], f32, tag="ps")
        for k in range(KT):
            nc.tensor.matmul(ps[:, :XS], wkvd_sb[:, k, mt * P:(mt + 1) * P],
                             xs[:, k, :], start=(k == 0), stop=(k == KT - 1))
        st = s1p.tile([P, XS], bf16, tag="s1e")
        nc.vector.tensor_copy(st[:], ps[:, :XS])
        nc.sync.dma_start(out=lat_kv_in[mt * P:(mt + 1) * P, :], in_=st[:])
    # k rope slice (dup across head slots) + rotation with per-core tables
    ps = psA.tile([P, CH], f32, tag="ps")
    for k in range(KT):
        nc.tensor.matmul(ps[:, :XS], wkr2_sb[:, k, :], xs[:, k, :],
                         start=(k == 0), stop=(k == KT - 1))
    kr_raw = ropep.tile([P, XS], bf16, tag="kr_raw")
    nc.vector.tensor_copy(kr_raw[:], ps[:, :XS])
    ps = psA.tile([P, CH], f32, tag="ps")
    for k in range(KT):
        nc.tensor.matmul(ps[:, :XS], wkr2p_sb[:, k, :], xs[:, k, :],
                         start=(k == 0), stop=(k == KT - 1))
    kr_rawp = ropep.tile([P, XS], bf16, tag="kr_rawp")
    nc.vector.tensor_copy(kr_rawp[:], ps[:, :XS])
    krot = ropep.tile([P, XS], bf16, tag="krot")
    rope_mix(krot[:], kr_raw, kr_rawp, cosk_sb, sink_sb, slice(0, XS))
    nc.sync.dma_start(out=lat_kv_in[KVR:KVR + P, :], in_=krot[:])

    nc.gpsimd.collective_compute(
        "AllGather", mybir.AluOpType.bypass,
        replica_groups=[list(range(NCORES))],
        ins=[lat_kv_in.opt()], outs=[lat_kv_out.opt()])

    # c_q slice (emitted after the kv collective so kv-side work starts first)
    for mt in range(RT):
        ps = psA.tile([P, CH], f32, tag="ps")
        for k in range(KT):
            nc.tensor.matmul(ps[:, :XS], wqd_sb[:, k, mt * P:(mt + 1) * P],
                             xs[:, k, :], start=(k == 0), stop=(k == KT - 1))
        st = s1p.tile([P, XS], bf16, tag="s1e")
        nc.vector.tensor_copy(st[:], ps[:, :XS])
        nc.sync.dma_start(out=lat_q_in[mt * P:(mt + 1) * P, :], in_=st[:])

    nc.gpsimd.collective_compute(
        "AllGather", mybir.AluOpType.bypass,
        replica_groups=[list(range(NCORES))],
        ins=[lat_q_in.opt()], outs=[lat_q_out.opt()])

    # gathered rotated k-rope, all positions
    nc.sync.dma_start(out=kTr2[:].rearrange("p (b n) -> p b n", b=NCORES),
                      in_=lat_kv_out[:, KVR:KVR + P, :].rearrange("b p n -> p b n"))

    # ---- stage 2: up-projections per s-chunk from gathered latents ----
    for sc in range(NCH):
        ssl = slice(sc * CH, (sc + 1) * CH)
        ckv = latp.tile([P, RT, CH], bf16, tag="ckv")
        for rt in range(RT):
            nc.sync.dma_start(
                out=ckv[:, rt, :].rearrange("p (b n) -> p b n", b=BPC),
                in_=lat_kv_out[sc * BPC:(sc + 1) * BPC, rt * P:(rt + 1) * P, :]
                .rearrange("b p n -> p b n"))
        # k content (d, h, t)
        for h in range(HC):
            ps = psA.tile([P, CH], f32, tag="ps")
            for k in range(RT):
                nc.tensor.matmul(ps[:], wku_sb[:, k, h * DV:(h + 1) * DV], ckv[:, k, :],
                                 start=(k == 0), stop=(k == RT - 1))
            nc.vector.tensor_copy(kTc[:, h, ssl], ps[:])
        # v (t, h*128+d) for the 4 t-tiles of this chunk
        for tt in range(CH // P):
            tg = sc * (CH // P) + tt
            ps = psA.tile([P, CH], f32, tag="ps")
            for k in range(RT):
                nc.tensor.matmul(ps[:, :HC * DV],
                                 ckv[:, k, tt * P:(tt + 1) * P],
                                 wvu_sb[:, k, :],
                                 start=(k == 0), stop=(k == RT - 1))
            nc.vector.tensor_copy(vv[:, tg, :], ps[:, :HC * DV])

        cq = latp.tile([P, RT, CH], bf16, tag="cq")
        for rt in range(RT):
            nc.sync.dma_start(
                out=cq[:, rt, :].rearrange("p (b n) -> p b n", b=BPC),
                in_=lat_q_out[sc * BPC:(sc + 1) * BPC, rt * P:(rt + 1) * P, :]
                .rearrange("b p n -> p b n"))
        # q content (d, h, s)
        for h in range(HC):
            ps = psA.tile([P, CH], f32, tag="ps")
            for k in range(RT):
                nc.tensor.matmul(ps[:], wqu_sb[:, k, h * DV:(h + 1) * DV], cq[:, k, :],
                                 start=(k == 0), stop=(k == RT - 1))
            nc.vector.tensor_copy(qTc[:, h, ssl], ps[:])
        # q rope (both heads at once: 128 rows) + rotation
        ps = psA.tile([P, CH], f32, tag="ps")
        for k in range(RT):
            nc.tensor.matmul(ps[:], wqr_sb[:, k, :], cq[:, k, :],
                             start=(k == 0), stop=(k == RT - 1))
        qr_raw = ropep.tile([P, CH], bf16, tag="qr_raw")
        nc.vector.tensor_copy(qr_raw[:], ps[:])
        ps = psA.tile([P, CH], f32, tag="ps")
        for k in range(RT):
            nc.tensor.matmul(ps[:], wqrp_sb[:, k, :], cq[:, k, :],
                             start=(k == 0), stop=(k == RT - 1))
        qr_rawp = ropep.tile([P, CH], bf16, tag="qr_rawp")
        nc.vector.tensor_copy(qr_rawp[:], ps[:])
        rope_mix(qTr[:, ssl], qr_raw, qr_rawp, cos_sb, sin_sb, ssl)

    # ---- attention, head-by-head, s-chunk by s-chunk ----
    HT = TT // 2    # half the t-tiles per pp buffer
    for h in range(HC):
        for sc in range(NCH):
            ssl = slice(sc * CH, (sc + 1) * CH)
            zps = psZ.tile([1, CH], f32, tag="z", name="zps")
            aps = psB.tile([P, CH], f32, tag="attn", name="aps")
            for th in range(2):
                pp = ppp.tile([P, HT, CH], bf16, tag="pp", name="pp")
                for ti in range(HT):
                    tt = th * HT + ti
                    ps = psA.tile([P, CH], f32, tag="ps", name="lg")
                    nc.tensor.matmul(ps[:], kTc[:, h, tt * P:(tt + 1) * P],
                                     qTc[:, h, ssl], start=True, stop=False)
                    nc.tensor.matmul(ps[:], kTr2[h * DR:(h + 1) * DR, tt * P:(tt + 1) * P],
                                     qTr[h * DR:(h + 1) * DR, ssl],
                                     start=False, stop=True)
                    nc.scalar.activation(pp[:, ti, :], ps[:],
                                         mybir.ActivationFunctionType.Exp, scale=SCALE)
                # pre-reduce exp'd tiles 2:1 twice on DVE, then the Z
                # ones-matmul only covers HT/4 tiles per half
                l1 = zlp.tile([P, HT // 2, CH], bf16, tag="zl1", name="zl1")
                nc.vector.tensor_add(l1[:], pp[:, 0:HT // 2, :], pp[:, HT // 2:HT, :])
                l2 = zlp.tile([P, HT // 4, CH], bf16, tag="zl2", name="zl2")
                nc.vector.tensor_add(l2[:], l1[:, 0:HT // 4, :], l1[:, HT // 4:HT // 2, :])
                for j in range(HT // 4):
                    nc.tensor.matmul(zps[:], ones[:], l2[:, j, :],
                                     start=(th == 0 and j == 0),
                                     stop=(th == 1 and j == HT // 4 - 1))
                for ti in range(HT):
                    tt = th * HT + ti
                    nc.tensor.matmul(aps[:], vv[:, tt, h * DV:(h + 1) * DV], pp[:, ti, :],
                                     start=(tt == 0), stop=(tt == TT - 1))
            zr = recp.tile([1, CH], f32, tag="zr")
            nc.vector.reciprocal(zr[:], zps[:])
            zrb = recp.tile([1, CH], bf16, tag="zrb")
            nc.vector.tensor_copy(zrb[:], zr[:])
            # broadcast recip-Z over partitions via a K=1 bf16 matmul
            zb_ps = psC.tile([P, CH], f32, tag="zb", name="zb_ps")
            nc.tensor.matmul(zb_ps[:], ones1f[:], zrb[:], start=True, stop=True)
            zb = recp.tile([P, CH], f32, tag="zb_sb")
            nc.vector.tensor_copy(zb[:], zb_ps[:])
            nc.vector.tensor_mul(attnT[:, h, ssl], aps[:], zb[:])

    # ---- out projection (partial over this core's heads) ----
    for ms in range(TT):
        osb = outp.tile([P, E], f32, tag="osb")
        for ec in range(NCH):
            ps = psA.tile([P, CH], f32, tag="ps")
            for h in range(HC):
                nc.tensor.matmul(ps[:], attnT[:, h, ms * P:(ms + 1) * P],
                                 wo_sb[:, h, ec * CH:(ec + 1) * CH],
                                 start=(h == 0), stop=(h == HC - 1))
            if (ms + ec) % 2 == 0:
                nc.scalar.copy(osb[:, ec * CH:(ec + 1) * CH], ps[:])
            else:
                nc.vector.tensor_copy(osb[:, ec * CH:(ec + 1) * CH], ps[:])
        nc.sync.dma_start(out=out[ms * P:(ms + 1) * P, :], in_=osb[:])


def _rope_tables():
    inv_freq = (1.0 / np.power(10000.0, np.arange(0, DR, 2, dtype=F32) / DR)).astype(F32)
    t = np.arange(S, dtype=F32)
    freqs = np.outer(t, inv_freq)          # (S, 32)
    sin, cos = np.sin(freqs), np.cos(freqs)
    # (d, s) layout for one head's 64 rope dims; sign folded for rotate-half
    c1 = np.concatenate([cos.T, cos.T], axis=0)            # (64, S)
    s1 = np.concatenate([-sin.T, sin.T], axis=0)
    cos2 = np.tile(c1, (HC, 1)).astype(BF16)               # (128, S)
    sin2 = np.tile(s1, (HC, 1)).astype(BF16)
    return cos2, sin2


_PERM64 = np.concatenate([np.arange(32, 64), np.arange(0, 32)])
_PERM128 = np.concatenate([_PERM64, 64 + _PERM64])


def _prep_in_maps(x, wq_down, wq_up, wq_rope, wkv_down, wkv_up, wk_rope, wo):
    cos2, sin2 = _rope_tables()
    wkrT2 = np.concatenate([wk_rope.T, wk_rope.T], axis=1)     # (E, 128)
    xT = np.ascontiguousarray(x.T).astype(BF16)
    shared = {
        "wqdT": np.ascontiguousarray(wq_down.T).astype(BF16),
        "wkvdT": np.ascontiguousarray(wkv_down.T).astype(BF16),
        "wkrT2": np.ascontiguousarray(wkrT2).astype(BF16),
        "wkrT2p": np.ascontiguousarray(wkrT2[:, _PERM128]).astype(BF16),
        "cos2": cos2,
        "sin2": sin2,
    }
    in_maps = []
    for c in range(NCORES):
        h0 = c * HC
        m = dict(shared)
        ssl = slice(c * XS, (c + 1) * XS)
        m["xTs"] = np.ascontiguousarray(xT[:, ssl])
        m["cosk"] = np.ascontiguousarray(cos2[:, ssl])
        m["sink"] = np.ascontiguousarray(sin2[:, ssl])
        wqrT = wq_rope[h0 * DR:(h0 + HC) * DR, :].T
        m["wquT"] = np.ascontiguousarray(wq_up[h0 * DV:(h0 + HC) * DV, :].T).astype(BF16)
        m["wqrT"] = np.ascontiguousarray(wqrT).astype(BF16)
        m["wqrTp"] = np.ascontiguousarray(wqrT[:, _PERM128]).astype(BF16)
        m["wkuT"] = np.ascontiguousarray(wkv_up[h0 * DV:(h0 + HC) * DV, :].T).astype(BF16)
        m["wvuT"] = np.ascontiguousarray(
            wkv_up[H * DV + h0 * DV:H * DV + (h0 + HC) * DV, :].T).astype(BF16)
        m["woT"] = np.ascontiguousarray(wo[:, h0 * DV:(h0 + HC) * DV].T).astype(BF16)
        in_maps.append(m)
    return in_maps


_NC_CACHE = {}


def _get_nc(repeat=1):
    if repeat not in _NC_CACHE:
        _NC_CACHE[repeat] = _build_nc(repeat)
    return _NC_CACHE[repeat]


def run(inputs, trace=False):
    nc = _get_nc()
    in_maps = _prep_in_maps(**{k: np.asarray(v, dtype=F32) for k, v in inputs.items()})
    res = run_bass_kernel_spmd(nc, in_maps, list(range(NCORES)), trace=trace)
    out = np.zeros((S, E), dtype=F32)
    for r in res.results:
        out += np.asarray(r["out"], dtype=F32)
    return out, res


def kernel(**inputs) -> np.ndarray:
    out, _ = run(inputs, trace=False)
    return out


# revision 53
# speedup vs baseline: 1.0270x; 1.0270x over previous
"""MLA (multi-head latent attention) Trainium2 kernel, 8-core SPMD.

Sharding: tensor-parallel over heads (16 heads / 8 cores = 2 heads per core).
The low-rank down-projections (c_q, c_kv, rotated k-rope) are computed
data-parallel over sequence positions (each core does its S/8 slice), then
all-gathered on-chip (two AllGather collectives so the kv-dependent
up-projections overlap the q-latent gather).  Up-projections, attention, and
the out-projection are head-sharded; each core produces a partial (S, E)
output (its 2 heads' contribution through wo) and the host sums the 8
partials.

All device matmuls run in bf16 with f32 PSUM accumulation.  Attention is
computed in transposed layout (logits[t, s], t on partitions) so that:
  - Q/K/V all come out of the projection matmuls in the right layout with no
    transposes anywhere,
  - softmax denominator Z[s] = sum_t exp(logits[t,s]) comes from two 2:1 DVE
    pre-reductions of the exp'd tiles followed by a short ones-vector matmul;
    1/Z is broadcast over partitions with a K=1 bf16 matmul,
  - exp is applied without max-subtraction (logits are O(6) here; reference
    softmax is mathematically identical).
RoPE's rotate-half permutation is folded into host-permuted copies of the
rope projection weights (an extra accumulation pass on the PE), and the
shared k-rope is duplicated across both head slots via host-duplicated
weights so per-head attention APs stay base-partition aligned.
"""

import sys
from contextlib import ExitStack

import numpy as np

sys.path.insert(0, "/opt/trn_rl_repo")

import ml_dtypes  # noqa: E402

import concourse.bass as bass  # noqa: E402
import concourse.tile as tile  # noqa: E402
from concourse import bacc  # noqa: E402
from concourse import mybir  # noqa: E402
from concourse.bass_utils import run_bass_kernel_spmd  # noqa: E402

BF16 = ml_dtypes.bfloat16
F32 = np.float32

# Model dims (hardcoded per problem spec)
E = 2048
H = 16
QR = 512
KVR = 512
DR = 64
DV = 128
S = 2048
NCORES = 8
HC = H // NCORES          # heads per core = 2
DQK = DV + DR             # 192
SCALE = 1.0 / float(np.sqrt(DQK))

P = 128                   # SBUF partitions
NCH = 4                   # s-chunks of 512
CH = S // NCH             # 512
KT = E // P               # 16 k-tiles over E
RT = QR // P              # 4 k-tiles over QR/KVR
TT = S // P               # 16 t-tiles over S
XS = S // NCORES          # per-core s-slice for the latent stage = 256
BPC = CH // XS            # gather blocks per s-chunk = 2

bf16 = mybir.dt.bfloat16
f32 = mybir.dt.float32


def _build_nc(repeat=1):
    nc = bacc.Bacc(None, target_bir_lowering=False)

    xTs = nc.declare_dram_parameter("xTs", [E, XS], bf16, isOutput=False)
    wqdT = nc.declare_dram_parameter("wqdT", [E, QR], bf16, isOutput=False)
    wkvdT = nc.declare_dram_parameter("wkvdT", [E, KVR], bf16, isOutput=False)
    # k-rope weights duplicated across both head slots (so per-head attention
    # APs stay base-partition aligned) + column-permuted copies (rotate-half)
    wkrT2 = nc.declare_dram_parameter("wkrT2", [E, HC * DR], bf16, isOutput=False)
    wkrT2p = nc.declare_dram_parameter("wkrT2p", [E, HC * DR], bf16, isOutput=False)
    wquT = nc.declare_dram_parameter("wquT", [QR, HC * DV], bf16, isOutput=False)
    wqrT = nc.declare_dram_parameter("wqrT", [QR, HC * DR], bf16, isOutput=False)
    wqrTp = nc.declare_dram_parameter("wqrTp", [QR, HC * DR], bf16, isOutput=False)
    wkuT = nc.declare_dram_parameter("wkuT", [KVR, HC * DV], bf16, isOutput=False)
    wvuT = nc.declare_dram_parameter("wvuT", [KVR, HC * DV], bf16, isOutput=False)
    woT = nc.declare_dram_parameter("woT", [HC * DV, E], bf16, isOutput=False)
    cos2 = nc.declare_dram_parameter("cos2", [HC * DR, S], bf16, isOutput=False)
    sin2 = nc.declare_dram_parameter("sin2", [HC * DR, S], bf16, isOutput=False)
    cosk = nc.declare_dram_parameter("cosk", [HC * DR, XS], bf16, isOutput=False)
    sink = nc.declare_dram_parameter("sink", [HC * DR, XS], bf16, isOutput=False)
    out = nc.declare_dram_parameter("out", [S, E], f32, isOutput=True)

    with tile.TileContext(nc) as tc, ExitStack() as ctx:
        const = ctx.enter_context(tc.tile_pool(name="const", bufs=1))
        wdp = ctx.enter_context(tc.tile_pool(name="wdown", bufs=1))
        wup = ctx.enter_context(tc.tile_pool(name="wup", bufs=1))
        xtp = ctx.enter_context(tc.tile_pool(name="xt", bufs=1))
        s1p = ctx.enter_context(tc.tile_pool(name="s1", bufs=2))
        latp = ctx.enter_context(tc.tile_pool(name="lat", bufs=2))
        pers = ctx.enter_context(tc.tile_pool(name="pers", bufs=1))
        ropep = ctx.enter_context(tc.tile_pool(name="rope", bufs=1))
        ppp = ctx.enter_context(tc.tile_pool(name="pp", bufs=2))
        zlp = ctx.enter_context(tc.tile_pool(name="zl", bufs=1))
        outp = ctx.enter_context(tc.tile_pool(name="outsb", bufs=1))
        recp = ctx.enter_context(tc.tile_pool(name="rec", bufs=2))
        dramp = ctx.enter_context(tc.tile_pool(name="dram", bufs=1, space="DRAM"))
        psA = ctx.enter_context(tc.tile_pool(name="psA", bufs=4, space="PSUM"))
        psB = ctx.enter_context(tc.tile_pool(name="psB", bufs=2, space="PSUM"))
        psZ = ctx.enter_context(tc.tile_pool(name="psZ", bufs=1, space="PSUM"))
        psC = ctx.enter_context(tc.tile_pool(name="psC", bufs=1, space="PSUM"))

        # --- constants ---
        ones = const.tile([P, 1], bf16)
        nc.any.memset(ones[:], 1.0)
        ones1f = const.tile([1, P], bf16)
        nc.any.memset(ones1f[:], 1.0)
        cos_sb = const.tile([P, S], bf16)
        nc.sync.dma_start(out=cos_sb[:], in_=cos2[:])
        sin_sb = const.tile([P, S], bf16)
        nc.sync.dma_start(out=sin_sb[:], in_=sin2[:])
        cosk_sb = const.tile([P, XS], bf16)
        nc.sync.dma_start(out=cosk_sb[:], in_=cosk[:])
        sink_sb = const.tile([P, XS], bf16)
        nc.sync.dma_start(out=sink_sb[:], in_=sink[:])

        # --- weights resident in SBUF ---
        wqd_sb = wdp.tile([P, KT, QR], bf16)
        nc.sync.dma_start(out=wqd_sb[:], in_=wqdT[:].rearrange("(k p) m -> p k m", p=P))
        wkvd_sb = wdp.tile([P, KT, KVR], bf16)
        nc.sync.dma_start(out=wkvd_sb[:], in_=wkvdT[:].rearrange("(k p) m -> p k m", p=P))
        wkr2_sb = wdp.tile([P, KT, HC * DR], bf16)
        nc.sync.dma_start(out=wkr2_sb[:], in_=wkrT2[:].rearrange("(k p) m -> p k m", p=P))
        wkr2p_sb = wdp.tile([P, KT, HC * DR], bf16)
        nc.sync.dma_start(out=wkr2p_sb[:], in_=wkrT2p[:].rearrange("(k p) m -> p k m", p=P))

        wqu_sb = wup.tile([P, RT, HC * DV], bf16)
        nc.sync.dma_start(out=wqu_sb[:], in_=wquT[:].rearrange("(k p) m -> p k m", p=P))
        wqr_sb = wup.tile([P, RT, HC * DR], bf16)
        nc.sync.dma_start(out=wqr_sb[:], in_=wqrT[:].rearrange("(k p) m -> p k m", p=P))
        wqrp_sb = wup.tile([P, RT, HC * DR], bf16)
        nc.sync.dma_start(out=wqrp_sb[:], in_=wqrTp[:].rearrange("(k p) m -> p k m", p=P))
        wku_sb = wup.tile([P, RT, HC * DV], bf16)
        nc.sync.dma_start(out=wku_sb[:], in_=wkuT[:].rearrange("(k p) m -> p k m", p=P))
        wvu_sb = wup.tile([P, RT, HC * DV], bf16)
        nc.sync.dma_start(out=wvu_sb[:], in_=wvuT[:].rearrange("(k p) m -> p k m", p=P))
        wo_sb = wup.tile([P, HC, E], bf16)
        nc.sync.dma_start(out=wo_sb[:], in_=woT[:].rearrange("(h p) e -> p h e", p=P))

        for _rep in range(repeat):
            _emit_iteration(
                nc, tc, ctx,
                xTs, out,
                ones, ones1f, cos_sb, sin_sb, cosk_sb, sink_sb,
                wqd_sb, wkvd_sb, wkr2_sb, wkr2p_sb,
                wqu_sb, wqr_sb, wqrp_sb, wku_sb, wvu_sb, wo_sb,
                xtp, s1p, latp, pers, ropep, ppp, zlp, outp, recp, dramp,
                psA, psB, psZ, psC,
            )

    nc.finalize()
    return nc


def _emit_iteration(nc, tc, ctx, xTs, out,
                    ones, ones1f, cos_sb, sin_sb, cosk_sb, sink_sb,
                    wqd_sb, wkvd_sb, wkr2_sb, wkr2p_sb,
                    wqu_sb, wqr_sb, wqrp_sb, wku_sb, wvu_sb, wo_sb,
                    xtp, s1p, latp, pers, ropep, ppp, zlp, outp, recp, dramp,
                    psA, psB, psZ, psC):
    # --- persistent activations (per iteration) ---
    qTc = pers.tile([P, HC, S], bf16, tag="qTc")    # q content, (d, h, s)
    qTr = pers.tile([P, S], bf16, tag="qTr")        # q rope rotated, (h*64+d, s)
    kTc = pers.tile([P, HC, S], bf16, tag="kTc")    # k content, (d, h, t)
    kTr2 = pers.tile([P, S], bf16, tag="kTr2")      # k rope rotated, dup per head
    vv = pers.tile([P, TT, HC * DV], bf16, tag="vv")  # v, (t%128, t//128, h*128+d)
    attnT = pers.tile([P, HC, S], bf16, tag="attnT")  # attention out, (d, h, s)

    lat_kv_in = dramp.tile([KVR + P, XS], bf16, tag="lkvi")
    lat_kv_out = dramp.tile([NCORES, KVR + P, XS], bf16, tag="lkvo",
                            addr_space="Shared")
    lat_q_in = dramp.tile([QR, XS], bf16, tag="lqi")
    lat_q_out = dramp.tile([NCORES, QR, XS], bf16, tag="lqo", addr_space="Shared")

    def rope_mix(dst, raw, rawp, csb, ssb, sl):
        """dst = raw * cos + perm(raw) * sin_signed (perm via rawp)."""
        n = sl.stop - sl.start
        t1 = ropep.tile([P, n], bf16, tag=f"ropet1_{n}")
        nc.vector.tensor_mul(t1[:], rawp[:], ssb[:, sl])
        t2 = ropep.tile([P, n], bf16, tag=f"ropet2_{n}")
        nc.vector.tensor_mul(t2[:], raw[:], csb[:, sl])
        nc.vector.tensor_add(dst, t1[:], t2[:])

    # ---- stage 1: latents for OWN s-slice, then all-gather ----
    xs = xtp.tile([P, KT, XS], bf16, tag="xs")
    nc.sync.dma_start(out=xs[:], in_=xTs[:].rearrange("(k p) n -> p k n", p=P))

    # c_kv slice
    for mt in range(RT):
        ps = psA.tile([P, CH], f32, tag="ps")
        for k in range(KT):
            nc.tensor.matmul(ps[:, :XS], wkvd_sb[:, k, mt * P:(mt + 1) * P],
                             xs[:, k, :], start=(k == 0), stop=(k == KT - 1))
        st = s1p.tile([P, XS], bf16, tag="s1e")
        nc.vector.tensor_copy(st[:], ps[:, :XS])
        nc.sync.dma_start(out=lat_kv_in[mt * P:(mt + 1) * P, :], in_=st[:])
    # k rope slice (dup across head slots) + rotation with per-core tables
    ps = psA.tile([P, CH], f32, tag="ps")
    for k in range(KT):
        nc.tensor.matmul(ps[:, :XS], wkr2_sb[:, k, :], xs[:, k, :],
                         start=(k == 0), stop=(k == KT - 1))
    kr_raw = ropep.tile([P, XS], bf16, tag="kr_raw")
    nc.vector.tensor_copy(kr_raw[:], ps[:, :XS])
    ps = psA.tile([P, CH], f32, tag="ps")
    for k in range(KT):
        nc.tensor.matmul(ps[:, :XS], wkr2p_sb[:, k, :], xs[:, k, :],
                         start=(k == 0), stop=(k == KT - 1))
    kr_rawp = ropep.tile([P, XS], bf16, tag="kr_rawp")
    nc.vector.tensor_copy(kr_rawp[:], ps[:, :XS])
    krot = ropep.tile([P, XS], bf16, tag="krot")
    rope_mix(krot[:], kr_raw, kr_rawp, cosk_sb, sink_sb, slice(0, XS))
    nc.sync.dma_start(out=lat_kv_in[KVR:KVR + P, :], in_=krot[:])

    nc.gpsimd.collective_compute(
        "AllGather", mybir.AluOpType.bypass,
        replica_groups=[list(range(NCORES))],
        ins=[lat_kv_in.opt()], outs=[lat_kv_out.opt()])

    # c_q slice (emitted after the kv collective so kv-side work starts first)
    for mt in range(RT):
        ps = psA.tile([P, CH], f32, tag="ps")
        for k in range(KT):
            nc.tensor.matmul(ps[:, :XS], wqd_sb[:, k, mt * P:(mt + 1) * P],
                             xs[:, k, :], start=(k == 0), stop=(k == KT - 1))
        st = s1p.tile([P, XS], bf16, tag="s1e")
        nc.vector.tensor_copy(st[:], ps[:, :XS])
        nc.sync.dma_start(out=lat_q_in[mt * P:(mt + 1) * P, :], in_=st[:])

    nc.gpsimd.collective_compute(
        "AllGather", mybir.AluOpType.bypass,
        replica_groups=[list(range(NCORES))],
        ins=[lat_q_in.opt()], outs=[lat_q_out.opt()])

    # gathered rotated k-rope, all positions
    nc.sync.dma_start(out=kTr2[:].rearrange("p (b n) -> p b n", b=NCORES),
                      in_=lat_kv_out[:, KVR:KVR + P, :].rearrange("b p n -> p b n"))

    # ---- stage 2: up-projections per s-chunk from gathered latents ----
    for sc in range(NCH):
        ssl = slice(sc * CH, (sc + 1) * CH)
        ckv = latp.tile([P, RT, CH], bf16, tag="ckv")
        for rt in range(RT):
            nc.sync.dma_start(
                out=ckv[:, rt, :].rearrange("p (b n) -> p b n", b=BPC),
                in_=lat_kv_out[sc * BPC:(sc + 1) * BPC, rt * P:(rt + 1) * P, :]
                .rearrange("b p n -> p b n"))
        # k content (d, h, t)
        for h in range(HC):
            ps = psA.tile([P, CH], f32, tag="ps")
            for k in range(RT):
                nc.tensor.matmul(ps[:], wku_sb[:, k, h * DV:(h + 1) * DV], ckv[:, k, :],
                                 start=(k == 0), stop=(k == RT - 1))
            nc.vector.tensor_copy(kTc[:, h, ssl], ps[:])
        # v (t, h*128+d) for the 4 t-tiles of this chunk
        for tt in range(CH // P):
            tg = sc * (CH // P) + tt
            ps = psA.tile([P, CH], f32, tag="ps")
            for k in range(RT):
                nc.tensor.matmul(ps[:, :HC * DV],
                                 ckv[:, k, tt * P:(tt + 1) * P],
                                 wvu_sb[:, k, :],
                                 start=(k == 0), stop=(k == RT - 1))
            nc.vector.tensor_copy(vv[:, tg, :], ps[:, :HC * DV])

        cq = latp.tile([P, RT, CH], bf16, tag="cq")
        for rt in range(RT):
            nc.sync.dma_start(
                out=cq[:, rt, :].rearrange("p (b n) -> p b n", b=BPC),
                in_=lat_q_out[sc * BPC:(sc + 1) * BPC, rt * P:(rt + 1) * P, :]
                .rearrange("b p n -> p b n"))
        # q content (d, h, s)
        for h in range(HC):
            ps = psA.tile([P, CH], f32, tag="ps")
            for k in range(RT):
                nc.tensor.matmul(ps[:], wqu_sb[:, k, h * DV:(h + 1) * DV], cq[:, k, :],
                                 start=(k == 0), stop=(k == RT - 1))
            nc.vector.tensor_copy(qTc[:, h, ssl], ps[:])
        # q rope (both heads at once: 128 rows) + rotation
        ps = psA.tile([P, CH], f32, tag="ps")
        for k in range(RT):
            nc.tensor.matmul(ps[:], wqr_sb[:, k, :], cq[:, k, :],
                             start=(k == 0), stop=(k == RT - 1))
        qr_raw = ropep.tile([P, CH], bf16, tag="qr_raw")
        nc.vector.tensor_copy(qr_raw[:], ps[:])
        ps = psA.tile([P, CH], f32, tag="ps")
        for k in range(RT):
            nc.tensor.matmul(ps[:], wqrp_sb[:, k, :], cq[:, k, :],
                             start=(k == 0), stop=(k == RT - 1))
        qr_rawp = ropep.tile([P, CH], bf16, tag="qr_rawp")
        nc.vector.tensor_copy(qr_rawp[:], ps[:])
        rope_mix(qTr[:, ssl], qr_raw, qr_rawp, cos_sb, sin_sb, ssl)

    # ---- attention, head-by-head, s-chunk by s-chunk ----
    HT = TT // 2    # half the t-tiles per pp buffer
    for h in range(HC):
        for sc in range(NCH):
            ssl = slice(sc * CH, (sc + 1) * CH)
            zps = psZ.tile([1, CH], f32, tag="z", name="zps")
            aps = psB.tile([P, CH], f32, tag="attn", name="aps")
            for th in range(2):
                pp = ppp.tile([P, HT, CH], bf16, tag="pp", name="pp")
                for ti in range(HT):
                    tt = th * HT + ti
                    ps = psA.tile([P, CH], f32, tag="ps", name="lg")
                    nc.tensor.matmul(ps[:], kTc[:, h, tt * P:(tt + 1) * P],
                                     qTc[:, h, ssl], start=True, stop=False)
                    nc.tensor.matmul(ps[:], kTr2[h * DR:(h + 1) * DR, tt * P:(tt + 1) * P],
                                     qTr[h * DR:(h + 1) * DR, ssl],
                                     start=False, stop=True)
                    nc.scalar.activation(pp[:, ti, :], ps[:],
                                         mybir.ActivationFunctionType.Exp, scale=SCALE)
                # pre-reduce exp'd tiles 2:1 twice on DVE, then the Z
                # ones-matmul only covers HT/4 tiles per half
                l1 = zlp.tile([P, HT // 2, CH], bf16, tag="zl1", name="zl1")
                nc.vector.tensor_add(l1[:], pp[:, 0:HT // 2, :], pp[:, HT // 2:HT, :])
                l2 = zlp.tile([P, HT // 4, CH], bf16, tag="zl2", name="zl2")
                nc.vector.tensor_add(l2[:], l1[:, 0:HT // 4, :], l1[:, HT // 4:HT // 2, :])
                for j in range(HT // 4):
                    nc.tensor.matmul(zps[:], ones[:], l2[:, j, :],
                                     start=(th == 0 and j == 0),
                                     stop=(th == 1 and j == HT // 4 - 1))
                for ti in range(HT):
                    tt = th * HT + ti
                    nc.tensor.matmul(aps[:], vv[:, tt, h * DV:(h + 1) * DV], pp[:, ti, :],
                                     start=(tt == 0), stop=(tt == TT - 1))
            zr = recp.tile([1, CH], f32, tag="zr")
            nc.vector.reciprocal(zr[:], zps[:])
            zrb = recp.tile([1, CH], bf16, tag="zrb")
            nc.vector.tensor_copy(zrb[:], zr[:])
            # broadcast recip-Z over partitions via a K=1 bf16 matmul
            zb_ps = psC.tile([P, CH], f32, tag="zb", name="zb_ps")
            nc.tensor.matmul(zb_ps[:], ones1f[:], zrb[:], start=True, stop=True)
            zb = recp.tile([P, CH], f32, tag="zb_sb")
            nc.vector.tensor_copy(zb[:], zb_ps[:])
            nc.vector.tensor_mul(attnT[:, h, ssl], aps[:], zb[:])

    # ---- out projection (partial over this core's heads) ----
    for ms in range(TT):
        osb = outp.tile([P, E], f32, tag="osb")
        for ec in range(NCH):
            ps = psA.tile([P, CH], f32, tag="ps")
            for h in range(HC):
                nc.tensor.matmul(ps[:], attnT[:, h, ms * P:(ms + 1) * P],
                                 wo_sb[:, h, ec * CH:(ec + 1) * CH],
                                 start=(h == 0), stop=(h == HC - 1))
            nc.scalar.copy(osb[:, ec * CH:(ec + 1) * CH], ps[:])
        nc.sync.dma_start(out=out[ms * P:(ms + 1) * P, :], in_=osb[:])


def _rope_tables():
    inv_freq = (1.0 / np.power(10000.0, np.arange(0, DR, 2, dtype=F32) / DR)).astype(F32)
    t = np.arange(S, dtype=F32)
    freqs = np.outer(t, inv_freq)          # (S, 32)
    sin, cos = np.sin(freqs), np.cos(freqs)
    # (d, s) layout for one head's 64 rope dims; sign folded for rotate-half
    c1 = np.concatenate([cos.T, cos.T], axis=0)            # (64, S)
    s1 = np.concatenate([-sin.T, sin.T], axis=0)
    cos2 = np.tile(c1, (HC, 1)).astype(BF16)               # (128, S)
    sin2 = np.tile(s1, (HC, 1)).astype(BF16)
    return cos2, sin2


_PERM64 = np.concatenate([np.arange(32, 64), np.arange(0, 32)])
_PERM128 = np.concatenate([_PERM64, 64 + _PERM64])


def _prep_in_maps(x, wq_down, wq_up, wq_rope, wkv_down, wkv_up, wk_rope, wo):
    cos2, sin2 = _rope_tables()
    wkrT2 = np.concatenate([wk_rope.T, wk_rope.T], axis=1)     # (E, 128)
    xT = np.ascontiguousarray(x.T).astype(BF16)
    shared = {
        "wqdT": np.ascontiguousarray(wq_down.T).astype(BF16),
        "wkvdT": np.ascontiguousarray(wkv_down.T).astype(BF16),
        "wkrT2": np.ascontiguousarray(wkrT2).astype(BF16),
        "wkrT2p": np.ascontiguousarray(wkrT2[:, _PERM128]).astype(BF16),
        "cos2": cos2,
        "sin2": sin2,
    }
    in_maps = []
    for c in range(NCORES):
        h0 = c * HC
        m = dict(shared)
        ssl = slice(c * XS, (c + 1) * XS)
        m["xTs"] = np.ascontiguousarray(xT[:, ssl])
        m["cosk"] = np.ascontiguousarray(cos2[:, ssl])
        m["sink"] = np.ascontiguousarray(sin2[:, ssl])
        wqrT = wq_rope[h0 * DR:(h0 + HC) * DR, :].T
        m["wquT"] = np.ascontiguousarray(wq_up[h0 * DV:(h0 + HC) * DV, :].T).astype(BF16)
        m["wqrT"] = np.ascontiguousarray(wqrT).astype(BF16)
        m["wqrTp"] = np.ascontiguousarray(wqrT[:, _PERM128]).astype(BF16)
        m["wkuT"] = np.ascontiguousarray(wkv_up[h0 * DV:(h0 + HC) * DV, :].T).astype(BF16)
        m["wvuT"] = np.ascontiguousarray(
            wkv_up[H * DV + h0 * DV:H * DV + (h0 + HC) * DV, :].T).astype(BF16)
        m["woT"] = np.ascontiguousarray(wo[:, h0 * DV:(h0 + HC) * DV].T).astype(BF16)
        in_maps.append(m)
    return in_maps


_NC_CACHE = {}


def _get_nc(repeat=1):
    if repeat not in _NC_CACHE:
        _NC_CACHE[repeat] = _build_nc(repeat)
    return _NC_CACHE[repeat]


def run(inputs, trace=False):
    nc = _get_nc()
    in_maps = _prep_in_maps(**{k: np.asarray(v, dtype=F32) for k, v in inputs.items()})
    res = run_bass_kernel_spmd(nc, in_maps, list(range(NCORES)), trace=trace)
    out = np.zeros((S, E), dtype=F32)
    for r in res.results:
        out += np.asarray(r["out"], dtype=F32)
    return out, res


def kernel(**inputs) -> np.ndarray:
    out, _ = run(inputs, trace=False)
    return out
